# revision 3
# baseline (speedup 1.0000x reference)
"""Trainium2 Bass kernel for nn_MinCostMatcher (focal-cls + L1 + GIoU matcher).

v2: engine-rebalanced rewrite of the baseline.
  - Two DVE `reciprocal` ops (4.0us each) -> `reciprocal_approx_fast`
    (~51 ULP, validated: min top-2 argmin margin on this data is 8.7e-4,
    pipeline-sim margin 4.3e-4, so approx error ~3e-6 rel cannot flip).
  - union gets a PE-side +1e-8 via an extra fp16 1e-4 x 1e-4 constant row in
    the U0 matmul, removing max(union,EPS) and making afast(union) safe
    (union' >= 1e-8 > 0 always). enc keeps a ts-max guard (enc==0 occurs).
  - Elementwise DAG trimmed to 22 ops/chunk, spread across DVE / ACT (Scalar)
    / Pool (GpSimd) so no single engine is ~80% busy like the baseline DVE.
  - Stage 1 writes the fp16 hi/lo focal-table planes to per-chunk DRAM
    scratch tensors in r-major row order (row t = r*128 + p, n = p*128 + r)
    so each xbar-transpose read depends only on its own chunk -> stage-1
    compute, scratch writes, transposes and the pairwise main loop pipeline.
    The argmax then yields a permuted index t; n = ((t&127)<<7) | (t>>7)
    is recovered at the end with 3 tiny int ops.

Layout/math otherwise as the baseline: per core = one batch element,
cls term via one-hot fp16 hi/lo matmul, pairwise rank-2 D_k matrices from PE,
GIoU via min/max identities, streaming top-1 argmax merge per 2048-col
super-chunk.
"""

import numpy as np
from contextlib import ExitStack

import concourse.bass as bass
import concourse.bacc as bacc
import concourse.tile as tile
from concourse import mybir
from concourse import bass_utils

F32 = mybir.dt.float32
F16 = mybir.dt.float16
I32 = mybir.dt.int32
U32 = mybir.dt.uint32
Alu = mybir.AluOpType
Act = mybir.ActivationFunctionType

B = 8
N = 16384
C = 80
M = 100
EPS = 1e-8
EPS_HALF = 1e-4     # fp16-safe; eps row product = fp16(1e-4)^2 ~ 1.0005e-8
NT = 512            # pairwise n-chunk (one PSUM bank)
NCH = N // NT       # 32
SUP = 2048          # argmax super-chunk
QF = 1280           # stage-1 flat free chunk (128 x 1280 = 16 rows x 80 comps)
QCH = (N * C) // (128 * QF)  # 8
RW = QF // C        # rows (within partition) per stage-1 chunk = 16
TR = RW * 128       # scratch rows per stage-1 chunk = 2048

# scratch column layout: 0-79 X_hi | 80 sp_hi | 81 sp_lo | 82,83 ones |
# 84+4k: pk_hi | 85+4k: pk_lo | 86+4k,87+4k: ones (k=0..3) |
# 100 pa_hi | 101 pa_lo | 102,103 ones | 104 eps_half | 105-127 pad
SCR_W = 128
NSMALL = 25
NEG_INF = -3.0e38


def emit_kernel(nc: bass.Bass, t: dict):
    cp = t["cp"].ap()        # (16384, 80) f32
    lp = t["lp"].ap()        # (16384, 4)  f32
    ct = t["ct"].ap()        # (100, 80)   f32
    lt = t["lt"].ap()        # (100, 4)    f32
    bidx = t["bidx"].ap()    # (100, 1)    i32
    scrS = t["scrS"].ap()    # (16, 100) f16 row-bounce scratch
    out = t["out"].ap()      # (100, 3)    i32
    # per-chunk scratch planes, rows t' = r'*128 + p (r-major)
    scrA = [t[f"scrA{j}"].ap() for j in range(QCH)]   # (2048, 128) f16
    scrB = [t[f"scrB{j}"].ap() for j in range(QCH)]

    cp3 = cp.rearrange("(p r) c -> p (r c)", p=128)      # (128, 10240)
    lp3 = lp.rearrange("(p r) c -> p r c", p=128)        # (128, 128, 4)
    # r-major views: [p, r', c] with DRAM row = r'*128 + p
    scrA3 = [a.rearrange("(r p) c -> p r c", p=128) for a in scrA]
    scrB3 = [a.rearrange("(r p) c -> p r c", p=128) for a in scrB]

    with tile.TileContext(nc) as tc, ExitStack() as ctx:
        singles = ctx.enter_context(tc.tile_pool(name="singles", bufs=1))
        eps_col = singles.tile([128, 1], F32)
        nc.vector.memset(eps_col, EPS)

        # transposed table reads land here as stage 1 streams through chunks
        xhT = singles.tile([128, N], F16)   # rows: scratch columns
        xlT = singles.tile([128, N], F16)
        # rhs row groups for D_k / U0 at matmul-legal bases (0/32/64 only,
        # and lhs/rhs bases must match); filled per-transpose-chunk below so
        # the main loop can stream behind stage 1.
        rhsD = singles.tile([68, N], F16)
        rhsD2 = singles.tile([37, N], F16)

        # ---------------- stage 1: per-n tables -> DRAM scratch -----------
        with tc.tile_pool(name="s1", bufs=6) as s1:
            # ---- loc tables (tiny) ----
            lpt = s1.tile([128, 128, 4], F32, tag="lp", bufs=8)
            nc.sync.dma_start(out=lpt, in_=lp3)
            lps = s1.tile([128, 128, 4], F32, tag="lp", bufs=8)
            nc.vector.tensor_scalar(lps, lpt, 1.0 / 128.0, None, Alu.mult)
            sp = s1.tile([128, 128], F32, tag="lp", bufs=8)
            nc.vector.tensor_reduce(sp, lps, axis=mybir.AxisListType.X, op=Alu.add)
            exty = s1.tile([128, 128], F32, tag="lp", bufs=8)
            nc.vector.tensor_tensor(exty, lps[:, :, 2], lps[:, :, 0], Alu.subtract)
            extx = s1.tile([128, 128], F32, tag="lp", bufs=8)
            nc.vector.tensor_tensor(extx, lps[:, :, 3], lps[:, :, 1], Alu.subtract)
            nc.vector.tensor_scalar(exty, exty, 0.0, None, Alu.max)
            nc.vector.tensor_scalar(extx, extx, 0.0, None, Alu.max)
            pa = s1.tile([128, 128], F32, tag="lp", bufs=8)
            nc.vector.tensor_tensor(pa, exty, extx, Alu.mult)

            # fp16 hi/lo splits -> small staging (scratch cols 80..104)
            small_st = s1.tile([128, 128, NSMALL], F16, tag="small", bufs=1)

            def split_to(dst_hi, dst_lo, src_f32):
                nc.vector.tensor_copy(out=dst_hi, in_=src_f32)
                lo32 = s1.tile([128, 128], F32, tag="lp", bufs=8)
                nc.vector.tensor_tensor(lo32, src_f32, dst_hi, Alu.subtract)
                nc.vector.tensor_copy(out=dst_lo, in_=lo32)

            split_to(small_st[:, :, 0], small_st[:, :, 1], sp)
            for k in range(4):
                split_to(small_st[:, :, 4 + 4 * k], small_st[:, :, 5 + 4 * k],
                         lps[:, :, k])
            split_to(small_st[:, :, 20], small_st[:, :, 21], pa)
            nc.vector.memset(small_st[:, :, 2:4], 1.0)
            for k in range(4):
                nc.vector.memset(small_st[:, :, 6 + 4 * k: 8 + 4 * k], 1.0)
            nc.vector.memset(small_st[:, :, 22:24], 1.0)
            nc.vector.memset(small_st[:, :, 24:25], EPS_HALF)
            zpad = s1.tile([128, RW, 48], F16, tag="zpad", bufs=1)
            nc.vector.memset(zpad, 0.0)

            # ---- focal table X = 0.75*p^2*ln(1-p+eps) - 0.25*(1-p)^2*ln(p+eps)
            # one r-slice (16 rows) per chunk; scratch writes + transpose per
            # chunk so the main loop can start after chunk 0.
            for j in range(QCH):
                sl = slice(j * QF, (j + 1) * QF)
                rsl = slice(j * RW, (j + 1) * RW)
                pj = s1.tile([128, QF], F32, tag="big")
                nc.gpsimd.dma_start(out=pj, in_=cp3[:, sl])
                qj = s1.tile([128, QF], F32, tag="big")   # 1-p
                nc.scalar.activation(qj, pj, Act.Identity, bias=1.0, scale=-1.0)
                ln1 = s1.tile([128, QF], F32, tag="big")  # ln(p+eps)
                nc.scalar.activation(ln1, pj, Act.Ln, bias=eps_col, scale=1.0)
                ln2 = s1.tile([128, QF], F32, tag="big")  # ln(1-p+eps)
                nc.scalar.activation(ln2, qj, Act.Ln, bias=eps_col, scale=1.0)
                sq = s1.tile([128, QF], F32, tag="big")    # 0.75*p^2
                nc.scalar.activation(sq, pj, Act.Square, scale=0.8660254037844386)
                sq1m = s1.tile([128, QF], F32, tag="big")  # 0.25*(1-p)^2
                nc.scalar.activation(sq1m, qj, Act.Square, scale=0.5)
                t2 = s1.tile([128, QF], F32, tag="big")    # 0.75*p^2*ln2
                nc.vector.tensor_tensor(t2, sq, ln2, Alu.mult)
                t3 = s1.tile([128, QF], F32, tag="big")    # 0.25*(1-p)^2*ln1
                nc.gpsimd.tensor_tensor(t3, sq1m, ln1, Alu.mult)
                xj2 = s1.tile([128, QF], F32, tag="big")
                nc.vector.tensor_tensor(xj2, t2, t3, Alu.subtract)
                xh = s1.tile([128, QF], F16, tag="bigh", bufs=4)
                nc.scalar.copy(out=xh, in_=xj2)
                lo32 = s1.tile([128, QF], F32, tag="big")
                nc.vector.tensor_tensor(lo32, xj2, xh, Alu.subtract)
                xl = s1.tile([128, QF], F16, tag="bigh", bufs=4)
                nc.vector.tensor_copy(out=xl, in_=lo32)
                # scratch writes (r-major rows), then per-chunk transpose
                nc.sync.dma_start(out=scrA3[j][:, :, 0:C],
                                  in_=xh.rearrange("p (r c) -> p r c", c=C))
                nc.sync.dma_start(out=scrB3[j][:, :, 0:C],
                                  in_=xl.rearrange("p (r c) -> p r c", c=C))
                nc.sync.dma_start(out=scrA3[j][:, :, 80:105],
                                  in_=small_st[:, rsl, :])
                nc.sync.dma_start(out=scrA3[j][:, :, 105:128],
                                  in_=zpad[:, :, 0:23])
                nc.sync.dma_start(out=scrB3[j][:, :, 80:128], in_=zpad)
                rs = slice(j * TR, (j + 1) * TR)
                nc.sync.dma_start_transpose(xhT[:, rs], scrA[j])
                nc.sync.dma_start_transpose(xlT[:, rs], scrB[j])
                nc.sync.dma_start(out=rhsD[0:4, rs], in_=xhT[84:88, rs])
                nc.sync.dma_start(out=rhsD[32:36, rs], in_=xhT[88:92, rs])
                nc.sync.dma_start(out=rhsD[64:68, rs], in_=xhT[92:96, rs])
                nc.sync.dma_start(out=rhsD2[0:4, rs], in_=xhT[96:100, rs])
                nc.sync.dma_start(out=rhsD2[32:37, rs], in_=xhT[100:105, rs])


        # ---------------- per-m scalars and lhsT weights ------------------
        ctt = singles.tile([M, C], F32)
        nc.sync.dma_start(out=ctt, in_=ct)
        ltt = singles.tile([M, 4], F32)
        nc.sync.dma_start(out=ltt, in_=lt)
        bcol = singles.tile([M, 1], I32)
        nc.sync.dma_start(out=bcol, in_=bidx)

        ht = singles.tile([M, 1], F32)
        nc.vector.tensor_tensor(ht, ltt[:, 2:3], ltt[:, 0:1], Alu.subtract)
        wt = singles.tile([M, 1], F32)
        nc.vector.tensor_tensor(wt, ltt[:, 3:4], ltt[:, 1:2], Alu.subtract)
        st = singles.tile([M, 1], F32)
        nc.vector.tensor_reduce(st, ltt, axis=mybir.AxisListType.X, op=Alu.add)
        rh = singles.tile([M, 1], F32)
        nc.vector.tensor_scalar(rh, ht, 0.0, None, Alu.max)
        rw = singles.tile([M, 1], F32)
        nc.vector.tensor_scalar(rw, wt, 0.0, None, Alu.max)
        ta = singles.tile([M, 1], F32)
        nc.vector.tensor_tensor(ta, rh, rw, Alu.mult)
        stp = singles.tile([M, 1], F32)   # +2.5*st (negated-G lhs row)
        nc.vector.tensor_scalar(stp, st, 2.5, None, Alu.mult)
        nht = singles.tile([M, 1], F32)   # -ht, -wt for ACT affine biases
        nc.vector.tensor_scalar(nht, ht, -1.0, None, Alu.mult)
        nwt = singles.tile([M, 1], F32)
        nc.vector.tensor_scalar(nwt, wt, -1.0, None, Alu.mult)

        def split_m(src, tag):  # (M,1) f32 -> fp16 (hi, lo)
            hi = singles.tile([M, 1], F16, tag=tag + "h")
            nc.vector.tensor_copy(out=hi, in_=src)
            lo32 = singles.tile([M, 1], F32, tag=tag + "l32")
            nc.vector.tensor_tensor(lo32, src, hi, Alu.subtract)
            lo = singles.tile([M, 1], F16, tag=tag + "l")
            nc.vector.tensor_copy(out=lo, in_=lo32)
            return hi, lo

        _row_ctr = [0]

        def to_row(dst_row_ap, col_f16):
            r = _row_ctr[0]
            _row_ctr[0] += 1
            nc.sync.dma_start(out=scrS[r:r + 1, :].rearrange("one m -> m one"),
                              in_=col_f16)
            nc.sync.dma_start(out=dst_row_ap, in_=scrS[r:r + 1, :])

        stp_hi, stp_lo = split_m(stp, "stp")
        ta_hi, ta_lo = split_m(ta, "ta")
        tk_splits = [split_m(ltt[:, k:k + 1], f"tk{k}") for k in range(4)]

        # lhsT for Gn = -G: rows 0-79 = -ct^T, 80,81 = -2.5, 82,83 = +2.5*st
        lhsG = singles.tile([84, M], F16)
        cttT = singles.tile([C, M], F32)
        nc.sync.dma_start(out=cttT, in_=ct.rearrange("m c -> c m"))
        nc.vector.tensor_scalar(lhsG[0:C, :], cttT, -1.0, None, Alu.mult)
        c25 = singles.tile([1, M], F16)
        nc.vector.memset(c25, -2.5)
        nc.sync.dma_start(out=lhsG[80:81, :], in_=c25)
        nc.sync.dma_start(out=lhsG[81:82, :], in_=c25)
        to_row(lhsG[82:83, :], stp_hi)
        to_row(lhsG[83:84, :], stp_lo)

        # lhsT rows for D_k / U0, packed to match rhsD/rhsD2 bases
        lhsDa = singles.tile([68, M], F16)
        lhsDb = singles.tile([37, M], F16)

        def fill_group(dst, rows_neg, hi, lo, neg_val):
            nc.vector.memset(dst[rows_neg], neg_val)
            to_row(dst[rows_neg.stop:rows_neg.stop + 1, :], hi)
            to_row(dst[rows_neg.stop + 1:rows_neg.stop + 2, :], lo)

        fill_group(lhsDa, slice(0, 2), *tk_splits[0], -1.0)    # y1
        fill_group(lhsDa, slice(32, 34), *tk_splits[1], -1.0)  # x1
        fill_group(lhsDa, slice(64, 66), *tk_splits[2], -1.0)  # y2
        fill_group(lhsDb, slice(0, 2), *tk_splits[3], -1.0)    # x2
        fill_group(lhsDb, slice(32, 34), ta_hi, ta_lo, 1.0)    # U0
        epsm = singles.tile([M, 1], F16)
        nc.vector.memset(epsm, EPS_HALF)
        to_row(lhsDb[36:37, :], epsm)                          # U0 += ~1e-8

        # ---------------- running argmax state -----------------------------
        bv = singles.tile([M, 1], F32)
        nc.vector.memset(bv, NEG_INF)
        bi = singles.tile([M, 1], U32)
        nc.vector.memset(bi, 0)

        # ---------------- pairwise main loop ------------------------------
        with tc.tile_pool(name="ps", bufs=8, space="PSUM") as ps, \
             tc.tile_pool(name="pw", bufs=30) as pw, \
             tc.tile_pool(name="nf", bufs=2) as nfp, \
             tc.tile_pool(name="mg", bufs=4) as mg:
            for js in range(N // SUP):
                NF = nfp.tile([M, SUP], F32, tag="nf")
                for jc in range(SUP // NT):
                    j = js * (SUP // NT) + jc
                    cs = slice(j * NT, (j + 1) * NT)
                    ls = slice(jc * NT, (jc + 1) * NT)

                    Gn = ps.tile([M, NT], F32, tag="psum")
                    nc.tensor.matmul(Gn, lhsG, xhT[0:84, cs], start=True, stop=False)
                    nc.tensor.matmul(Gn, lhsG[0:C, :], xlT[0:C, cs],
                                     start=False, stop=True)

                    Dy1 = ps.tile([M, NT], F32, tag="psum")
                    nc.tensor.matmul(Dy1, lhsDa[0:4, :], rhsD[0:4, cs],
                                     start=True, stop=True)
                    Dx1 = ps.tile([M, NT], F32, tag="psum")
                    nc.tensor.matmul(Dx1, lhsDa[32:36, :], rhsD[32:36, cs],
                                     start=True, stop=True)
                    Dy2 = ps.tile([M, NT], F32, tag="psum")
                    nc.tensor.matmul(Dy2, lhsDa[64:68, :], rhsD[64:68, cs],
                                     start=True, stop=True)
                    Dx2 = ps.tile([M, NT], F32, tag="psum")
                    nc.tensor.matmul(Dx2, lhsDb[0:4, :], rhsD2[0:4, cs],
                                     start=True, stop=True)
                    U0 = ps.tile([M, NT], F32, tag="psum")
                    nc.tensor.matmul(U0, lhsDb[32:37, :], rhsD2[32:37, cs],
                                     start=True, stop=True)

                    _tn = [0]

                    def T(tag="t"):
                        _tn[0] += 1
                        return pw.tile([M, NT], F32, tag=tag,
                                       name=f"pw{j}_{_tn[0]}")

                    # GPSIMD can't read PSUM and only runs plain tt/ts, so
                    # ACT produces relus + affine-shifted copies (a=D1+hw,
                    # b=D2-hw) from PSUM; Pool runs the u/d/e tt chains; DVE
                    # keeps PSUM-coupled union/t1, reciprocals, and the tail.
                    r1y = T()
                    nc.scalar.activation(r1y, Dy1, Act.Relu)
                    r2y = T()
                    nc.scalar.activation(r2y, Dy2, Act.Relu)
                    r1x = T()
                    nc.scalar.activation(r1x, Dx1, Act.Relu)
                    r2x = T()
                    nc.scalar.activation(r2x, Dx2, Act.Relu)
                    a_y = T()
                    nc.scalar.activation(a_y, Dy1, Act.Identity, bias=ht)
                    a_x = T()
                    nc.scalar.activation(a_x, Dx1, Act.Identity, bias=wt)
                    b_y = T()
                    nc.scalar.activation(b_y, Dy2, Act.Identity, bias=nht)
                    b_x = T()
                    nc.scalar.activation(b_x, Dx2, Act.Identity, bias=nwt)

                    # u = relu(D1)+relu(D2); d = (D1+hw)-u; e = u-(D2-hw)
                    u_y = T()
                    nc.gpsimd.tensor_tensor(u_y, r1y, r2y, Alu.add)
                    u_x = T()
                    nc.gpsimd.tensor_tensor(u_x, r1x, r2x, Alu.add)
                    d_y = T()
                    nc.gpsimd.tensor_tensor(d_y, a_y, u_y, Alu.subtract)
                    d_x = T()
                    nc.gpsimd.tensor_tensor(d_x, a_x, u_x, Alu.subtract)
                    e_y = T()
                    nc.gpsimd.tensor_tensor(e_y, u_y, b_y, Alu.subtract)
                    e_x = T()
                    nc.gpsimd.tensor_tensor(e_x, u_x, b_x, Alu.subtract)

                    iw = T()
                    nc.scalar.activation(iw, d_x, Act.Relu)
                    inter = T()
                    nc.vector.scalar_tensor_tensor(inter, d_y, 0.0, iw,
                                                   Alu.max, Alu.mult)
                    enc = T()
                    nc.gpsimd.tensor_tensor(enc, e_y, e_x, Alu.mult)
                    union = T()
                    nc.vector.tensor_tensor(union, U0, inter, Alu.subtract)

                    urcp = T()
                    nc.vector.reciprocal_approx_fast(out=urcp, in_=union)
                    iou = T()
                    nc.vector.tensor_tensor(iou, inter, urcp, Alu.mult)

                    gnum = T()
                    nc.vector.tensor_tensor(gnum, enc, union, Alu.subtract)
                    eden = T()
                    nc.vector.tensor_scalar(eden, enc, EPS, None, Alu.max)
                    ercp = T()
                    nc.vector.reciprocal_approx_fast(out=ercp, in_=eden)
                    gterm = T()
                    nc.vector.scalar_tensor_tensor(gterm, gnum, 0.0, ercp,
                                                   Alu.max, Alu.mult)

                    acc1 = T()
                    nc.vector.tensor_tensor(acc1, iou, gterm, Alu.subtract)
                    sr = T()
                    nc.vector.tensor_tensor(sr, u_y, u_x, Alu.add)
                    t1 = T()
                    nc.vector.scalar_tensor_tensor(t1, sr, -5.0, Gn,
                                                   Alu.mult, Alu.add)
                    nc.vector.tensor_tensor(NF[:, ls], acc1, t1, Alu.add)

                # ---- super-chunk argmax + running top-1 merge ----
                mx8 = mg.tile([M, 8], F32, tag="mx")
                nc.vector.max(mx8, NF)
                ix8 = mg.tile([M, 8], U32, tag="ix")
                nc.vector.max_index(ix8, mx8, NF)
                cmp = mg.tile([M, 1], U32, tag="cmp")
                nc.vector.tensor_tensor(cmp, mx8[:, 0:1], bv, Alu.is_gt)
                nc.vector.tensor_tensor(bv, bv, mx8[:, 0:1], Alu.max)
                ixg = mg.tile([M, 1], U32, tag="ixg")
                nc.vector.tensor_scalar(ixg, ix8[:, 0:1], js * SUP, None, Alu.add)
                nc.vector.copy_predicated(out=bi, mask=cmp, data=ixg)

        # ---------------- index remap + cls_id + output -------------------
        # scratch row t = r*128+p holds n = p*128+r -> n = ((t&127)<<7)|(t>>7)
        t_lo = singles.tile([M, 1], U32)
        nc.vector.tensor_scalar(t_lo, bi, 127, 7, Alu.bitwise_and,
                                Alu.logical_shift_left)
        t_hi = singles.tile([M, 1], U32)
        nc.vector.tensor_scalar(t_hi, bi, 7, None, Alu.logical_shift_right)
        n_idx = singles.tile([M, 1], U32)
        nc.vector.tensor_tensor(n_idx, t_lo, t_hi, Alu.add)

        cmx8 = singles.tile([M, 8], F32)
        nc.vector.max(cmx8, ctt)
        cix8 = singles.tile([M, 8], U32)
        nc.vector.max_index(cix8, cmx8, ctt)

        outc = singles.tile([M, 3], I32)
        nc.vector.tensor_copy(out=outc[:, 0:1], in_=bcol)
        nc.vector.tensor_copy(out=outc[:, 1:2], in_=n_idx)
        nc.vector.tensor_copy(out=outc[:, 2:3], in_=cix8[:, 0:1])
        nc.sync.dma_start(out=out, in_=outc)

    return nc


def build_nc():
    nc = bacc.Bacc("TRN2", target_bir_lowering=False, debug=False)
    t = {}
    t["cp"] = nc.dram_tensor("cp", (N, C), F32, kind="ExternalInput")
    t["lp"] = nc.dram_tensor("lp", (N, 4), F32, kind="ExternalInput")
    t["ct"] = nc.dram_tensor("ct", (M, C), F32, kind="ExternalInput")
    t["lt"] = nc.dram_tensor("lt", (M, 4), F32, kind="ExternalInput")
    t["bidx"] = nc.dram_tensor("bidx", (M, 1), I32, kind="ExternalInput")
    for j in range(QCH):
        t[f"scrA{j}"] = nc.dram_tensor(f"scrA{j}", (TR, SCR_W), F16,
                                       kind="Internal")
        t[f"scrB{j}"] = nc.dram_tensor(f"scrB{j}", (TR, SCR_W), F16,
                                       kind="Internal")
    t["scrS"] = nc.dram_tensor("scrS", (16, M), F16, kind="Internal")
    t["out"] = nc.dram_tensor("out", (M, 3), I32, kind="ExternalOutput")
    emit_kernel(nc, t)
    nc.finalize()
    return nc


_NC_CACHE = None


def kernel(cls_pred, loc_pred, cls_true, loc_true, reg_mask=None):
    global _NC_CACHE
    if _NC_CACHE is None:
        _NC_CACHE = build_nc()
    nc = _NC_CACHE

    b, w, h, c = cls_pred.shape
    assert (b, w * h, c) == (B, N, C)
    in_maps = []
    for i in range(B):
        in_maps.append({
            "cp": np.ascontiguousarray(cls_pred[i].reshape(N, C), np.float32),
            "lp": np.ascontiguousarray(loc_pred[i].reshape(N, 4), np.float32),
            "ct": np.ascontiguousarray(cls_true[i], np.float32),
            "lt": np.ascontiguousarray(loc_true[i], np.float32),
            "bidx": np.full((M, 1), i, np.int32),
        })
    res = bass_utils.run_bass_kernel_spmd(nc, in_maps, core_ids=list(range(B)))
    outs = [r["out"].reshape(M, 3) for r in res.results]
    return np.stack(outs, axis=0).astype(np.int32)


if __name__ == "__main__":
    import reference
    inputs = reference.setup_inputs()
    inputs = {k: np.asarray(v) for k, v in inputs.items()}
    got = kernel(**inputs)
    print(got[0, :5])
